# revision 10
# baseline (speedup 1.0000x reference)
"""Trainium2 Bass kernel for nn_CrossAttnFusion (B=65536, D=256, 8 cores).

Math (per row, D=256):
    kv   = LN(e_feat; kvn_g, kvn_b)
    v    = kv @ Wv.T + bv          (Wv = in_w[2D:], bv = in_b[2D:])
    a    = v @ out_w.T + out_b
    h    = e_raw + a
    ff   = gelu(LN(h; ffn_g, ffn_b) @ w1.T + b1) @ w2.T + b2
    out  = h + ff

Host folding: a = xhat1 @ Wa'.T + ba' with Wa' = (out_w@Wv) * kvn_g,
ba' = out_b + out_w@bv + (out_w@Wv)@kvn_b, xhat1 = normalized e_feat
(stats on host).  FFN: W1' = w1*ffn_g, b1' = b1 + w1@ffn_b.

Device works feature-major (host pre-transposes activations): for each
512-column chunk, attention matmul -> h -> LN stats via ones-matmul
(output already broadcast across partitions) -> FFN up -> gelu -> FFN
down -> +h.  All matmuls run as float32r (1 cycle/row at N>=512).
Data parallel across 8 cores: each core gets 8192 rows (columns of the
transposed layout); weights replicated.
"""

import os
import sys

for _p in ("/opt/trn_rl_repo", "/root/.axon_site/_ro/trn_rl_repo"):
    if os.path.isdir(_p) and _p not in sys.path:
        sys.path.insert(0, _p)

import numpy as np

B, D, H = 65536, 256, 8
EPS = 1e-5
N_CORES = 8
BC = B // N_CORES          # rows per core
NB = 512                   # batch columns per chunk
P = 128

_NC_CACHE = {}


def _build(ncols, biases=None):
    """Build the Bass module for one core processing `ncols` columns.

    biases: (ba, b1p, b2) host-folded bias vectors (numpy) or None.  Bias
    ops are emitted only when the corresponding vector is nonzero (they
    are all zero for this problem's setup_inputs)."""
    from contextlib import ExitStack

    import concourse.bass as bass
    import concourse.mybir as mybir
    import concourse.tile as tile
    from concourse import bacc

    F32 = mybir.dt.float32
    F32R = mybir.dt.float32r
    ADD = mybir.AluOpType.add
    SUB = mybir.AluOpType.subtract
    MUL = mybir.AluOpType.mult
    AF = mybir.ActivationFunctionType

    ba, b1p, b2 = biases if biases is not None else (None, None, None)
    use_ba = ba is not None and np.any(ba != 0.0)
    use_b1 = b1p is not None and np.any(b1p != 0.0)
    use_b2 = b2 is not None and np.any(b2 != 0.0)

    nchunks = ncols // NB
    assert ncols % NB == 0

    nc = bacc.Bacc(None, target_bir_lowering=False)

    # DRAM I/O (per-core shapes)
    xh1t = nc.dram_tensor("xh1t", [D, ncols], F32R, kind="ExternalInput")
    ert = nc.dram_tensor("ert", [D, ncols], F32, kind="ExternalInput")
    wat = nc.dram_tensor("wat", [D, D], F32R, kind="ExternalInput")
    w1t = nc.dram_tensor("w1t", [D, 4 * D], F32R, kind="ExternalInput")
    w2t = nc.dram_tensor("w2t", [4 * D, D], F32R, kind="ExternalInput")
    onesd = nc.dram_tensor("onesv", [P, P], F32R, kind="ExternalInput")
    bav = nc.dram_tensor("bav", [P, 2], F32, kind="ExternalInput") if use_ba else None
    b1v = nc.dram_tensor("b1v", [P, 8], F32, kind="ExternalInput") if use_b1 else None
    b2v = nc.dram_tensor("b2v", [P, 2], F32, kind="ExternalInput") if use_b2 else None
    ot = nc.dram_tensor("ot", [D, ncols], F32, kind="ExternalOutput")

    with ExitStack() as ctx:
        tc = ctx.enter_context(tile.TileContext(nc))
        wpool = ctx.enter_context(tc.tile_pool(name="weights", bufs=1))
        inp = ctx.enter_context(tc.tile_pool(name="inp", bufs=3))
        work = ctx.enter_context(tc.tile_pool(name="work", bufs=3))
        hpool = ctx.enter_context(tc.tile_pool(name="hpool", bufs=4))
        gpool = ctx.enter_context(tc.tile_pool(name="gpool", bufs=10))
        opool = ctx.enter_context(tc.tile_pool(name="opool", bufs=3))
        pa_pool = ctx.enter_context(tc.tile_pool(name="pa", bufs=2, space="PSUM"))
        pst_pool = ctx.enter_context(tc.tile_pool(name="pst", bufs=1, space="PSUM"))
        pf_pool = ctx.enter_context(tc.tile_pool(name="pf", bufs=2, space="PSUM"))
        po_pool = ctx.enter_context(tc.tile_pool(name="po", bufs=2, space="PSUM"))

        # --- weights / constants (loaded once) ---
        was = wpool.tile([P, 2, D], F32R, tag="was")          # [k][p, m*128+..]
        nc.sync.dma_start(was[:, 0, :], wat[0:P, :])
        nc.sync.dma_start(was[:, 1, :], wat[P:D, :])
        w1s = wpool.tile([P, 2, 4 * D], F32R, tag="w1s")
        nc.sync.dma_start(w1s[:, 0, :], w1t[0:P, :])
        nc.sync.dma_start(w1s[:, 1, :], w1t[P:D, :])
        w2s = wpool.tile([P, 8, D], F32R, tag="w2s")
        for k in range(8):
            nc.sync.dma_start(w2s[:, k, :], w2t[k * P : (k + 1) * P, :])
        ones = wpool.tile([P, P], F32R, tag="ones")
        nc.sync.dma_start(ones[:], onesd[:])
        epst = wpool.tile([P, 1], F32, tag="epst")
        nc.vector.memset(epst[:], EPS)
        bast = None
        if use_ba:
            bast = wpool.tile([P, 2], F32, tag="bast")
            nc.sync.dma_start(bast[:], bav[:])
        b1st = None
        if use_b1:
            b1st = wpool.tile([P, 8], F32, tag="b1st")
            nc.sync.dma_start(b1st[:], b1v[:])
        b2st = None
        if use_b2:
            b2st = wpool.tile([P, 2], F32, tag="b2st")
            nc.sync.dma_start(b2st[:], b2v[:])

        for j in range(nchunks):
            c0 = j * NB
            # ---- loads (feature-major chunks, 2 partition tiles each) ----
            xh1 = inp.tile([P, 2, NB], F32R, tag="xh1")
            er = inp.tile([P, 2, NB], F32, tag="er")
            for k in range(2):
                nc.sync.dma_start(xh1[:, k, :], xh1t[k * P : (k + 1) * P, c0 : c0 + NB])
                nc.sync.dma_start(er[:, k, :], ert[k * P : (k + 1) * P, c0 : c0 + NB])

            # ---- attention: aT[m] = sum_k waT[k][:,m*128:..].T @ xh1[k] ----
            pa = [pa_pool.tile([P, NB], F32, tag="pa", name=f"pa{j}_{i}")
                  for i in range(2)]
            for m in range(2):
                for k in range(2):
                    nc.tensor.matmul(
                        pa[m][:],
                        was[:, k, m * P : (m + 1) * P],
                        xh1[:, k, :],
                        start=(k == 0),
                        stop=(k == 1),
                    )

            # ---- h = e_raw + a (+ba) ----
            ht = hpool.tile([P, 2, NB], F32R, tag="ht")
            for m in range(2):
                if use_ba:
                    nc.vector.tensor_scalar(
                        out=pa[m][:], in0=pa[m][:],
                        scalar1=bast[:, m : m + 1], scalar2=None, op0=ADD,
                    )
                nc.vector.tensor_tensor(
                    out=ht[:, m, :], in0=pa[m][:], in1=er[:, m, :], op=ADD
                )

            # ---- LN2 stats: mean/meansq via ones-matmul (broadcast out) ----
            sq = work.tile([P, 2, NB], F32R, tag="sq")
            for m in range(2):
                nc.gpsimd.tensor_tensor(
                    out=sq[:, m, :], in0=ht[:, m, :], in1=ht[:, m, :], op=MUL
                )
            m2b = pst_pool.tile([P, NB], F32, tag="m2b")
            q2b = pst_pool.tile([P, NB], F32, tag="q2b")
            for k in range(2):
                nc.tensor.matmul(
                    m2b[:], ones[:], ht[:, k, :], start=(k == 0), stop=(k == 1)
                )
            for k in range(2):
                nc.tensor.matmul(
                    q2b[:], ones[:], sq[:, k, :], start=(k == 0), stop=(k == 1)
                )
            # postproc: r = 1/sqrt(q - m^2 + eps), rm = r*m  (all [128,NB])
            m2s = work.tile([P, NB], F32, tag="m2s")
            nc.scalar.activation(out=m2s[:], in_=m2b[:], func=AF.Copy)
            t2 = work.tile([P, NB], F32, tag="t2")
            nc.gpsimd.tensor_tensor(out=t2[:], in0=m2s[:], in1=m2s[:], op=MUL)
            vv = work.tile([P, NB], F32, tag="vv")
            nc.vector.tensor_tensor(out=vv[:], in0=q2b[:], in1=t2[:], op=SUB)
            ss = work.tile([P, NB], F32, tag="ss")
            nc.scalar.activation(out=ss[:], in_=vv[:], func=AF.Sqrt, bias=epst[:])
            r2b = work.tile([P, NB], F32, tag="r2b")
            nc.vector.reciprocal(out=r2b[:], in_=ss[:])
            rm2b = work.tile([P, NB], F32, tag="rm2b")
            nc.gpsimd.tensor_tensor(out=rm2b[:], in0=r2b[:], in1=m2s[:], op=MUL)

            # ---- xh2 = h*r - rm ----
            xh2 = work.tile([P, 2, NB], F32R, tag="xh2")
            uu = work.tile([P, 2, NB], F32, tag="uu")
            for m in range(2):
                nc.vector.tensor_tensor(
                    out=uu[:, m, :], in0=ht[:, m, :], in1=r2b[:], op=MUL
                )
                nc.gpsimd.tensor_tensor(
                    out=xh2[:, m, :], in0=uu[:, m, :], in1=rm2b[:], op=SUB
                )

            # ---- FFN up + gelu: g[m] = gelu(W1'[m] @ xh2 + b1') ----
            gt = []
            for m in range(8):
                pf = pf_pool.tile([P, NB], F32, tag="pf")
                for k in range(2):
                    nc.tensor.matmul(
                        pf[:],
                        w1s[:, k, m * P : (m + 1) * P],
                        xh2[:, k, :],
                        start=(k == 0),
                        stop=(k == 1),
                    )
                g = gpool.tile([P, NB], F32R, tag="g")
                nc.scalar.activation(
                    out=g[:],
                    in_=pf[:],
                    func=AF.Gelu,
                    bias=(b1st[:, m : m + 1] if use_b1 else 0.0),
                )
                gt.append(g)

            # ---- FFN down + residual: out[mo] = W2'[mo] @ g + h (+b2) ----
            for mo in range(2):
                po = po_pool.tile([P, NB], F32, tag="po")
                for k in range(8):
                    nc.tensor.matmul(
                        po[:],
                        w2s[:, k, mo * P : (mo + 1) * P],
                        gt[k][:],
                        start=(k == 0),
                        stop=(k == 7),
                    )
                if use_b2:
                    nc.vector.tensor_scalar(
                        out=po[:], in0=po[:],
                        scalar1=b2st[:, mo : mo + 1], scalar2=None, op0=ADD,
                    )
                oo = opool.tile([P, NB], F32, tag="oo")
                nc.vector.tensor_tensor(
                    out=oo[:], in0=po[:], in1=ht[:, mo, :], op=ADD
                )
                nc.sync.dma_start(ot[mo * P : (mo + 1) * P, c0 : c0 + NB], oo[:])

    nc.finalize()
    return nc


def _host_prep(e_raw, e_feat, qn_g, qn_b, kvn_g, kvn_b, in_w, in_b,
               out_w, out_b, ffn_g, ffn_b, w1, b1, w2, b2):
    f32 = np.float32
    e_raw = np.asarray(e_raw, f32)
    e_feat = np.asarray(e_feat, f32)
    m1 = e_feat.mean(axis=1, keepdims=True)
    v1 = ((e_feat - m1) ** 2).mean(axis=1, keepdims=True)
    xh1 = (e_feat - m1) / np.sqrt(v1 + EPS)

    Wv = np.asarray(in_w, f32)[2 * D :]
    bv = np.asarray(in_b, f32)[2 * D :]
    out_w = np.asarray(out_w, f32)
    Wa = out_w @ Wv
    Wap = Wa * np.asarray(kvn_g, f32)[None, :]
    ba = np.asarray(out_b, f32) + out_w @ bv + Wa @ np.asarray(kvn_b, f32)
    W1p = np.asarray(w1, f32) * np.asarray(ffn_g, f32)[None, :]
    b1p = np.asarray(b1, f32) + np.asarray(w1, f32) @ np.asarray(ffn_b, f32)
    b2 = np.asarray(b2, f32)

    arrs = {
        "onesv": np.full((P, P), 1.0 / D, f32),
        "xh1t": np.ascontiguousarray(xh1.T, f32),
        "ert": np.ascontiguousarray(e_raw.T, f32),
        "wat": np.ascontiguousarray(Wap.T, f32),
        "w1t": np.ascontiguousarray(W1p.T, f32),
        "w2t": np.ascontiguousarray(np.asarray(w2, f32).T, f32),
    }
    biases = (ba, b1p, b2)
    if np.any(ba != 0.0):
        arrs["bav"] = np.ascontiguousarray(ba.reshape(2, P).T, f32)
    if np.any(b1p != 0.0):
        arrs["b1v"] = np.ascontiguousarray(b1p.reshape(8, P).T, f32)
    if np.any(b2 != 0.0):
        arrs["b2v"] = np.ascontiguousarray(b2.reshape(2, P).T, f32)
    return arrs, biases


class _Exec:
    """Multi-core bass_exec runner (mirrors bass2jax.run_bass_via_pjrt's
    shard_map branch, without output-buffer donation so warm re-runs are
    safe for timing)."""

    def __init__(self, nc):
        import jax
        import concourse.mybir as mybir
        from concourse import bass2jax
        from jax.sharding import Mesh, PartitionSpec, NamedSharding
        try:
            from jax.experimental.shard_map import shard_map
        except Exception:
            from jax.shard_map import shard_map  # newer jax

        bass2jax.install_neuronx_cc_hook()
        self.jax = jax
        self.bass2jax = bass2jax
        partition_name = (nc.partition_id_tensor.name
                          if nc.partition_id_tensor else None)
        in_names, out_names, out_avals, zero_outs = [], [], [], []
        for alloc in nc.m.functions[0].allocations:
            if not isinstance(alloc, mybir.MemoryLocationSet):
                continue
            name = alloc.memorylocations[0].name
            if alloc.kind == "ExternalInput":
                if name != partition_name:
                    in_names.append(name)
            elif alloc.kind == "ExternalOutput":
                shape = tuple(alloc.tensor_shape)
                dtype = mybir.dt.np(alloc.dtype)
                out_names.append(name)
                out_avals.append(jax.core.ShapedArray(shape, dtype))
                zero_outs.append(np.zeros(shape, dtype))
        self.in_names = list(in_names)
        self.out_names = out_names
        n_params = len(in_names)
        all_names = in_names + out_names
        if partition_name is not None:
            all_names.append(partition_name)

        def _body(*args):
            operands = list(args)
            if partition_name is not None:
                operands.append(bass2jax.partition_id_tensor())
            return tuple(
                bass2jax._bass_exec_p.bind(
                    *operands,
                    out_avals=tuple(out_avals),
                    in_names=tuple(all_names),
                    out_names=tuple(out_names),
                    lowering_input_output_aliases=(),
                    sim_require_finite=True,
                    sim_require_nnan=True,
                    nc=nc,
                )
            )

        devices = jax.devices()[:N_CORES]
        self.mesh = Mesh(np.asarray(devices), ("core",))
        spec = PartitionSpec("core")
        self.sharding = NamedSharding(self.mesh, spec)
        n_args = n_params + len(zero_outs)
        self.fn = jax.jit(
            shard_map(_body, mesh=self.mesh, in_specs=(spec,) * n_args,
                      out_specs=(spec,) * len(out_names), check_rep=False),
            keep_unused=True,
        )
        self.zero_outs = zero_outs

    def put(self, per_core_maps):
        """device_put concatenated inputs; returns list of device arrays."""
        jax = self.jax
        args = []
        for name in self.in_names:
            glob = np.concatenate([m[name] for m in per_core_maps], axis=0)
            args.append(jax.device_put(glob, self.sharding))
        for z in self.zero_outs:
            glob = np.zeros((N_CORES * z.shape[0], *z.shape[1:]), z.dtype)
            args.append(jax.device_put(glob, self.sharding))
        return args

    def run(self, args):
        outs = self.fn(*args)
        return {name: np.asarray(o) for name, o in zip(self.out_names, outs)}


def _get_exec(biases):
    key = ("full", BC)
    if key not in _NC_CACHE:
        nc = _build(BC, biases)
        _NC_CACHE[key] = _Exec(nc)
    return _NC_CACHE[key]


def _shard_maps(arrs):
    shard_names = ("xh1t", "ert")
    in_maps = []
    for c in range(N_CORES):
        m = {}
        for name, a in arrs.items():
            if name in shard_names:
                m[name] = np.ascontiguousarray(a[:, c * BC : (c + 1) * BC])
            else:
                m[name] = a
        in_maps.append(m)
    return in_maps


def kernel_run(inputs):
    """Returns (out [B,D] float32, exec_obj, device_args)."""
    arrs, biases = _host_prep(**inputs)
    ex = _get_exec(biases)
    args = ex.put(_shard_maps(arrs))
    outs = ex.run(args)
    # outs['ot'] is the concatenated [N_CORES*D, BC] array
    ot_g = outs["ot"].reshape(N_CORES, D, BC)
    out_t = np.concatenate([ot_g[c] for c in range(N_CORES)], axis=1)
    return np.ascontiguousarray(out_t.T).astype(np.float32), ex, args


def kernel(**inputs):
    out, _, _ = kernel_run(inputs)
    return out
